# revision 1
# baseline (speedup 1.0000x reference)
"""LookupConv2d Trainium2 kernel.

Math: out = conv2d(x, W), W[o] = sum_s coeff[o,s] * dictionary[idx[o,s]].
Factorization: W = M @ D where M[o,d] = sum_{s: idx[o,s]=d} coeff[o,s] is a
(512, 100) scatter of the coefficients.  Then
    out = M @ conv2d(x, dictionary)
i.e. a 100-channel conv (23 GFLOP) followed by a 1x1 512x100 mix (5 GFLOP)
instead of a 512-channel conv (118 GFLOP) -- 4.2x fewer FLOPs.

Precision: the TensorE f32r mode streams 1 row/cycle (4x faster than fp32
mode) but rounds operands to 12 significant bits (RNE, measured on HW).
We split x and the dictionary into exact 12-bit halves (xh = top 12 bits,
xl = remainder, both f32r-invariant) and accumulate
    xh*wh + xl*wh + xh*wl
in fp32 PSUM -- full fp32-class accuracy (only xl*wl ~ 2^-24 dropped) at
3 cycles/row instead of fp32 mode's 4.  The small 1x1 mix stays in native
fp32 mode.

Sharding: data-parallel over batch N=16 -> 2 images per core on 8 cores.
dictionary (as [128,100] lhsT tap matrices) and M^T are replicated.
"""

import numpy as np

N_CORES = 8
IMGS_PER_CORE = 2
CIN = 256
COUT = 512
NDICT = 100
H = W = 56
HP = WP = 58  # padded
ROWS_PER_TILE = 8
N_TILES = H // ROWS_PER_TILE  # 7
FREE = ROWS_PER_TILE * W  # 448
S = 3  # lookup sparsity

TRACE = False  # set by test.py to get a profile
_LAST_RESULTS = {}  # test.py reads exec_time_ns from here


def split12(a):
    """Exact split a = hi + lo with <=12 significant bits each (a ~ N(0,1),
    so no denormal/overflow concerns).  Both halves pass through the f32r
    12-bit RNE rounding unchanged."""
    a = np.ascontiguousarray(a, dtype=np.float32)
    hi = (a.view(np.uint32) & np.uint32(0xFFFFF000)).view(np.float32)
    lo = (a - hi).astype(np.float32)
    return hi, lo


def _build_program():
    import concourse.bacc as bacc
    import concourse.mybir as mybir
    import concourse.tile as tile

    f32 = mybir.dt.float32
    f32r = mybir.dt.float32r

    nc = bacc.Bacc("TRN2", target_bir_lowering=False, debug=False)

    xh_d = nc.dram_tensor("xh", (IMGS_PER_CORE, CIN, HP, WP), f32,
                          kind="ExternalInput")
    xl_d = nc.dram_tensor("xl", (IMGS_PER_CORE, CIN, HP, WP), f32,
                          kind="ExternalInput")
    wh_d = nc.dram_tensor("wh", (128, 2 * 9 * NDICT), f32, kind="ExternalInput")
    wl_d = nc.dram_tensor("wl", (128, 2 * 9 * NDICT), f32, kind="ExternalInput")
    mh_d = nc.dram_tensor("mh", (NDICT, COUT), f32, kind="ExternalInput")
    ml_d = nc.dram_tensor("ml", (NDICT, COUT), f32, kind="ExternalInput")
    out_d = nc.dram_tensor("out", (IMGS_PER_CORE, COUT, H, W), f32,
                           kind="ExternalOutput")

    # row chunks of the padded input: first 10 rows, then 6x8 -- tile t only
    # needs chunks 0..t so compute starts after the first chunk lands
    row_chunks = [(0, 10)] + [(10 + 8 * k, 8) for k in range(6)]

    with tile.TileContext(nc) as tc:
        with (
            tc.tile_pool(name="consts", bufs=1) as consts,
            tc.tile_pool(name="xpool", bufs=1) as xpool,
            tc.tile_pool(name="ypool", bufs=3) as ypool,
            tc.tile_pool(name="opool", bufs=8) as opool,
            tc.tile_pool(name="psum_y", bufs=2, space="PSUM") as psum_y_pool,
            tc.tile_pool(name="psum_o", bufs=4, space="PSUM") as psum_o_pool,
        ):
            wh_sb = consts.tile([128, 2 * 9 * NDICT], f32r)
            nc.sync.dma_start(wh_sb[:], wh_d[:].bitcast(f32r))
            wl_sb = consts.tile([128, 2 * 9 * NDICT], f32r)
            nc.sync.dma_start(wl_sb[:], wl_d[:].bitcast(f32r))
            mh_sb = consts.tile([NDICT, COUT], f32r)
            nc.sync.dma_start(mh_sb[:], mh_d[:].bitcast(f32r))
            ml_sb = consts.tile([NDICT, COUT], f32r)
            nc.sync.dma_start(ml_sb[:], ml_d[:].bitcast(f32r))

            # [128 cin-in-block, img, cblk, hp, wp]
            xh_sb = xpool.tile([128, IMGS_PER_CORE, 2, HP, WP], f32r,
                               tag="xh_sb")
            xl_sb = xpool.tile([128, IMGS_PER_CORE, 2, HP, WP], f32r,
                               tag="xl_sb")
            xh_v = xh_d.rearrange("i (b c) h w -> c i b h w", c=128)
            xl_v = xl_d.rearrange("i (b c) h w -> c i b h w", c=128)
            for img in range(IMGS_PER_CORE):
                for r0, nr in row_chunks:
                    for cb in range(2):
                        nc.sync.dma_start(
                            xh_sb[:, img, cb, r0:r0 + nr, :],
                            xh_v[:, img, cb, r0:r0 + nr, :].bitcast(f32r))
                        nc.sync.dma_start(
                            xl_sb[:, img, cb, r0:r0 + nr, :],
                            xl_v[:, img, cb, r0:r0 + nr, :].bitcast(f32r))

            out_v = out_d.rearrange("i (b o) h w -> i b o (h w)", o=128)

            n_mm = 3 * 18

            def emit_conv(img, h0):
                py = psum_y_pool.tile([NDICT, FREE], f32)
                k = 0
                for cb in range(2):
                    for ti in range(3):
                        for tj in range(3):
                            tap = slice((cb * 9 + ti * 3 + tj) * NDICT,
                                        (cb * 9 + ti * 3 + tj + 1) * NDICT)
                            rh = (slice(None), img, cb,
                                  slice(h0 + ti, h0 + ti + ROWS_PER_TILE),
                                  slice(tj, tj + W))
                            for lhsT, rhs in (
                                (wh_sb[:, tap], xh_sb[rh]),
                                (wh_sb[:, tap], xl_sb[rh]),
                                (wl_sb[:, tap], xh_sb[rh]),
                            ):
                                nc.tensor.matmul(
                                    py[:], lhsT, rhs,
                                    start=(k == 0), stop=(k == n_mm - 1))
                                k += 1
                return py

            def emit_mix(py, img, h0):
                # Veltkamp split y = yh + yl into 12-bit halves (pure fp32
                # arithmetic; values are exactly f32r-representable so the
                # matmul's internal rounding is the identity)
                t_sb = ypool.tile([NDICT, FREE], f32, tag="t")
                big = ypool.tile([NDICT, FREE], f32, tag="big")
                yh = ypool.tile([NDICT, FREE], f32r, tag="yh")
                yl = ypool.tile([NDICT, FREE], f32r, tag="yl")
                nc.scalar.mul(t_sb[:], py[:], 4097.0)
                nc.vector.tensor_sub(big[:], t_sb[:], py[:])
                nc.vector.tensor_sub(yh[:], t_sb[:], big[:])
                nc.vector.tensor_sub(yl[:], py[:], yh[:])
                for ob in range(4):
                    obs = slice(ob * 128, (ob + 1) * 128)
                    po = psum_o_pool.tile([128, FREE], f32)
                    nc.tensor.matmul(po[:], mh_sb[:, obs], yh[:],
                                     start=True, stop=False)
                    nc.tensor.matmul(po[:], ml_sb[:, obs], yh[:],
                                     start=False, stop=False)
                    nc.tensor.matmul(po[:], mh_sb[:, obs], yl[:],
                                     start=False, stop=True)
                    o_sb = opool.tile([128, FREE], f32)
                    if ob % 2 == 0:
                        nc.vector.tensor_copy(o_sb[:], po[:])
                    else:
                        nc.scalar.copy(o_sb[:], po[:])
                    nc.sync.dma_start(
                        out_v[img, ob, :, h0 * W:h0 * W + FREE], o_sb[:])

            # software-pipeline by one tile: PE runs tile i's conv while
            # ACT/DVE run tile i-1's Veltkamp split, so the mix matmuls are
            # ready when PE gets to them
            pending = None
            for img in range(IMGS_PER_CORE):
                for t in range(N_TILES):
                    h0 = t * ROWS_PER_TILE
                    py = emit_conv(img, h0)
                    if pending is not None:
                        emit_mix(*pending)
                    pending = (py, img, h0)
            emit_mix(*pending)

    nc.compile()
    return nc


_NC_CACHE = None


def kernel(x, dictionary, lookup_indices, lookup_coefficients):
    global _NC_CACHE
    from concourse import bass_utils

    x = np.asarray(x, dtype=np.float32)
    dictionary = np.asarray(dictionary, dtype=np.float32)
    idx = np.asarray(lookup_indices).astype(np.int64)
    coef = np.asarray(lookup_coefficients, dtype=np.float32)

    # M^T[d, o] = sum_s coeff[o, s] * [idx[o, s] == d]
    mt = np.zeros((NDICT, COUT), np.float32)
    np.add.at(mt, (idx.reshape(-1),
                   np.repeat(np.arange(COUT), S)), coef.reshape(-1))

    # wt[c_in_block, (cblk, ti, tj, d)] = dictionary[d, cblk*128+c, ti, tj]
    wt = np.ascontiguousarray(
        dictionary.reshape(NDICT, 2, 128, 3, 3).transpose(2, 1, 3, 4, 0)
    ).reshape(128, 2 * 9 * NDICT)
    wh, wl = split12(wt)
    mh, ml = split12(mt)

    xp = np.pad(x, ((0, 0), (0, 0), (1, 1), (1, 1)))
    xp = np.ascontiguousarray(
        xp.reshape(N_CORES, IMGS_PER_CORE, CIN, HP, WP))
    xh, xl = split12(xp)

    if _NC_CACHE is None:
        _NC_CACHE = _build_program()
    nc = _NC_CACHE

    in_maps = [{"xh": xh[i], "xl": xl[i], "wh": wh, "wl": wl,
                "mh": mh, "ml": ml} for i in range(N_CORES)]
    try:
        res = bass_utils.run_bass_kernel_spmd(
            nc, in_maps, core_ids=list(range(N_CORES)), trace=TRACE)
    except ModuleNotFoundError:
        # no axon NTFF profile hook in this environment
        res = bass_utils.run_bass_kernel_spmd(
            nc, in_maps, core_ids=list(range(N_CORES)), trace=False)
    _LAST_RESULTS["res"] = res

    out = np.concatenate([r["out"] for r in res.results], axis=0)
    return out.reshape(16, COUT, H, W)



# revision 45
# speedup vs baseline: 2.8763x; 2.8763x over previous
"""LookupConv2d Trainium2 kernel.

Math: out = conv2d(x, W), W[o] = sum_s coeff[o,s] * dictionary[idx[o,s]].
Factorization: W = M @ D where M[o,d] = sum_{s: idx[o,s]=d} coeff[o,s] is a
(512, 100) scatter of the coefficients.  Then
    out = M @ conv2d(x, dictionary)
i.e. a 100-channel conv (23 GFLOP) followed by a 1x1 512x100 mix (5 GFLOP)
instead of a 512-channel conv (118 GFLOP) -- 4.2x fewer FLOPs.

Precision: single-pass bf16 throughout (1 row/cycle on the PE, same rate as
f32r but half the SBUF/DMA bytes).  Accumulation is fp32 in PSUM.  Measured
end-to-end rel err vs the fp32 reference is ~3.6e-3, comfortably inside the
2e-2 gate; the output is stored bf16 and widened to fp32 on the host.

Schedule: software-pipelined conv(i) / mix(i-1) on the PE.  First and last
tiles are 2 rows tall so the DMA-latency chains (launch+transfer+sem, about
2.2us each way) off the critical path shrink.  Input chunk DMAs and the
batched per-tile output DMAs ride the SP HWDGE queue; weights ride ACT.

Sharding: data-parallel over batch N=16 -> 2 images per core on 8 cores.
The dictionary tap matrices and M^T are small and replicated.
"""

import numpy as np

N_CORES = 8
IMGS_PER_CORE = 2
CIN = 256
COUT = 512
NDICT = 100
H = W = 56
HP = WP = 58  # padded
FREE = 8 * W  # 448, psum tile free size (max tile)
S = 3  # lookup sparsity

# (h0, rows) per image, in global processing order.  img0 runs top-to-bottom
# with full 8-row tiles (a big first tile keeps the tap-consumption cadence
# slower than the weight DMAs land) but its last 8 rows are deferred to the
# very end as two small tiles; img1 does its bottom tiles early.  The final
# four tiles are then all small, so every large output DMA fires while the
# PE is still computing and the program drain is one short chain.
TILES0 = [(8 * t, 8) for t in range(6)]
TILES1 = ([(48, 6), (54, 2)]
          + [(0, 8), (8, 8), (16, 8), (24, 8), (32, 8)]
          + [(40, 6), (46, 2)])
TILES_END0 = [(48, 6), (54, 2)]  # img0's deferred bottom rows
# input row chunks (r0, nr) per image
CHUNKS0 = [(10, 12), (22, 12), (34, 12), (46, 12)]
CHUNKS1 = [(46, 12), (0, 16), (16, 16), (32, 14)]

TRACE = False  # set by test.py to get a profile
_LAST_RESULTS = {}  # test.py reads exec_time_ns from here


def _build_program():
    import concourse.bacc as bacc
    import concourse.mybir as mybir
    import concourse.tile as tile

    f32 = mybir.dt.float32
    bf16 = mybir.dt.bfloat16

    nc = bacc.Bacc("TRN2", target_bir_lowering=False, debug=False)

    # host pre-arranges x into the SBUF layout [cin_in_block, img, cblk, h, w]
    x_d = nc.dram_tensor("x", (128, IMGS_PER_CORE, 2, HP, WP), bf16,
                         kind="ExternalInput")
    w_d = nc.dram_tensor("w", (128, 2 * 9 * NDICT), bf16, kind="ExternalInput")
    m_d = nc.dram_tensor("m", (NDICT, COUT), bf16, kind="ExternalInput")
    # [o_in_block, img, ob, h*w]; host transposes to (i, 512, 56, 56)
    out_d = nc.dram_tensor("out", (128, IMGS_PER_CORE, 4, H * W), bf16,
                           kind="ExternalOutput")

    with tile.TileContext(nc) as tc:
        with (
            tc.tile_pool(name="consts", bufs=1) as consts,
            tc.tile_pool(name="xpool", bufs=1) as xpool,
            tc.tile_pool(name="ypool", bufs=6) as ypool,
            tc.tile_pool(name="opool", bufs=6) as opool,
            tc.tile_pool(name="psum_y", bufs=2, space="PSUM") as psum_y_pool,
            tc.tile_pool(name="psum_o", bufs=6, space="PSUM") as psum_o_pool,
        ):
            # startup-critical loads take the fastest paths: the first three
            # taps ride SWDGE (no HWDGE queueing), the first cb0 x rows are
            # first in the SP HWDGE queue, the rest of the weights ride ACT
            w_sb = consts.tile([128, 2 * 9 * NDICT], bf16)
            x_sb = xpool.tile([128, IMGS_PER_CORE, 2, HP, WP], bf16,
                              tag="x_sb")
            # startup: the first conv runs full-width taps (187ns cadence) so
            # the weight stream keeps ahead: taps 0-8 are the first ACT-queue
            # transfer, taps 9-17 the second, interleaved with the SP queue's
            # first x rows (cb0 needed at tap 0, cb1 at tap 9).
            nc.gpsimd.dma_start(w_sb[:, :6 * NDICT], w_d[:, :6 * NDICT])
            nc.sync.dma_start(x_sb[:, 0, 0, 0:10, :], x_d[:, 0, 0, 0:10, :])
            nc.scalar.dma_start(w_sb[:, 6 * NDICT:], w_d[:, 6 * NDICT:])
            nc.sync.dma_start(x_sb[:, 0, 1, 0:10, :], x_d[:, 0, 1, 0:10, :])
            m_sb = consts.tile([NDICT, COUT], bf16)
            nc.scalar.dma_start(m_sb[:], m_d[:])

            # PE p-state warmup: the Tensor engine clocks 1.2GHz until it has
            # been busy ~3us.  Burn the startup DMA window with tiny matmuls
            # on a zeroed scratch tile so real work starts at full clock.
            scratch = consts.tile([128, 64], bf16)
            nc.vector.memset(scratch[:], 0.0)
            wpsum = psum_y_pool.tile([NDICT, FREE], f32, tag="py")
            for _ in range(46):
                nc.tensor.matmul(wpsum[:64, :64], scratch[:], scratch[:],
                                 start=True, stop=True)

            for img, chunks in ((0, CHUNKS0), (1, CHUNKS1)):
                for r0, nr in chunks:
                    nc.sync.dma_start(
                        x_sb[:, img, :, r0:r0 + nr, :],
                        x_d[:, img, :, r0:r0 + nr, :])

            def emit_conv(img, h0, rows, split=True):
                """Two half-tiles in one psum tile; each half's y copy is
                emitted right after its 18 taps, so the first half's copy
                overlaps the second half's matmuls and the mix never stalls
                on the psum->sbuf latency (a stall also resets the PE
                p-state, costing ~2us)."""
                ft = rows * W
                py = psum_y_pool.tile([NDICT, FREE], f32, tag="py")
                y_sb = ypool.tile([NDICT, FREE], bf16, tag="y")
                halves = [(0, rows)] if (rows <= 2 or not split) else [
                    (0, rows // 2), (rows // 2, rows - rows // 2)]
                for hi, (ro, hr) in enumerate(halves):
                    c0, c1 = ro * W, (ro + hr) * W
                    for k in range(18):
                        cb, t9 = divmod(k, 9)
                        ti, tj = divmod(t9, 3)
                        tap = slice(k * NDICT, (k + 1) * NDICT)
                        rh = (slice(None), img, cb,
                              slice(h0 + ro + ti, h0 + ro + ti + hr),
                              slice(tj, tj + W))
                        nc.tensor.matmul(
                            py[:, c0:c1], w_sb[:, tap], x_sb[rh],
                            start=(k == 0), stop=(k == 17))
                    if hi == 0:
                        nc.vector.tensor_copy(y_sb[:, c0:c1], py[:, c0:c1])
                    else:
                        nc.scalar.copy(y_sb[:, c0:c1], py[:, c0:c1])
                return py, y_sb

            def emit_mix(py, y_sb, img, h0, rows, dma_eng=None, tail=False):
                ft = rows * W
                o_sb = opool.tile([128, 4, FREE], bf16, tag="o")
                for ob in range(4):
                    obs = slice(ob * 128, (ob + 1) * 128)
                    po = psum_o_pool.tile([128, FREE], f32, tag="po")
                    nc.tensor.matmul(po[:, :ft], m_sb[:, obs], y_sb[:, :ft],
                                     start=True, stop=True)
                    # only DVE and ACT can read PSUM (GPSIMD cannot)
                    if ob % 2 == 0:
                        nc.vector.tensor_copy(o_sb[:, ob, :ft], po[:, :ft])
                    else:
                        nc.scalar.copy(o_sb[:, ob, :ft], po[:, :ft])
                # batched output DMA, on the (mostly idle) SP queue except
                # where the caller spreads the tail across queues
                (dma_eng or nc.sync).dma_start(
                    out_d[:, img, :, h0 * W:h0 * W + ft], o_sb[:, :, :ft])

            # software-pipeline by one tile: PE runs tile i's conv while the
            # copy engines drain tile i-1's psum, so the mix matmuls are
            # ready when PE gets to them
            all_tiles = ([(0,) + t for t in TILES0]
                         + [(1,) + t for t in TILES1]
                         + [(0,) + t for t in TILES_END0])
            n_total = len(all_tiles)
            pending = None
            for i, (img, h0, rows) in enumerate(all_tiles):
                py, y_sb = emit_conv(img, h0, rows, split=(i > 0))
                if pending is not None:
                    # spread the last output DMAs across both HWDGE queues
                    # so the drain chains overlap
                    eng = nc.scalar if i == n_total - 1 else None
                    emit_mix(*pending, dma_eng=eng, tail=(i >= n_total - 2))
                pending = (py, y_sb, img, h0, rows)
            emit_mix(*pending, tail=True)

    nc.compile()
    return nc


_NC_CACHE = None


def kernel(x, dictionary, lookup_indices, lookup_coefficients):
    global _NC_CACHE
    import ml_dtypes
    from concourse import bass_utils

    bf16 = ml_dtypes.bfloat16

    x = np.asarray(x, dtype=np.float32)
    dictionary = np.asarray(dictionary, dtype=np.float32)
    idx = np.asarray(lookup_indices).astype(np.int64)
    coef = np.asarray(lookup_coefficients, dtype=np.float32)

    # M^T[d, o] = sum_s coeff[o, s] * [idx[o, s] == d]
    mt = np.zeros((NDICT, COUT), np.float32)
    np.add.at(mt, (idx.reshape(-1),
                   np.repeat(np.arange(COUT), S)), coef.reshape(-1))

    # wt[c_in_block, (cblk, ti, tj, d)] = dictionary[d, cblk*128+c, ti, tj]
    wt = np.ascontiguousarray(
        dictionary.reshape(NDICT, 2, 128, 3, 3).transpose(2, 1, 3, 4, 0)
    ).reshape(128, 2 * 9 * NDICT)

    # x -> [core, cin_in_block, img, cblk, hp, wp] (the SBUF layout, so the
    # device DMA is a plain contiguous copy)
    xp = np.pad(x, ((0, 0), (0, 0), (1, 1), (1, 1)))
    xp = xp.reshape(N_CORES, IMGS_PER_CORE, 2, 128, HP, WP)
    xp = np.ascontiguousarray(xp.transpose(0, 3, 1, 2, 4, 5))

    if _NC_CACHE is None:
        _NC_CACHE = _build_program()
    nc = _NC_CACHE

    in_maps = [{"x": xp[i].astype(bf16), "w": wt.astype(bf16),
                "m": mt.astype(bf16)} for i in range(N_CORES)]
    try:
        res = bass_utils.run_bass_kernel_spmd(
            nc, in_maps, core_ids=list(range(N_CORES)), trace=TRACE)
    except ModuleNotFoundError:
        # no axon NTFF profile hook in this environment
        res = bass_utils.run_bass_kernel_spmd(
            nc, in_maps, core_ids=list(range(N_CORES)), trace=False)
    _LAST_RESULTS["res"] = res

    # [core, o_in_block, img, ob, hw] -> (16, 512, 56, 56) fp32
    out = np.stack([r["out"] for r in res.results], axis=0)
    out = out.astype(np.float32).transpose(0, 2, 3, 1, 4)
    return np.ascontiguousarray(out).reshape(16, COUT, H, W)


# revision 54
# speedup vs baseline: 2.9315x; 1.0192x over previous
"""LookupConv2d Trainium2 kernel.

Math: out = conv2d(x, W), W[o] = sum_s coeff[o,s] * dictionary[idx[o,s]].
Factorization: W = M @ D where M[o,d] = sum_{s: idx[o,s]=d} coeff[o,s] is a
(512, 100) scatter of the coefficients.  Then
    out = M @ conv2d(x, dictionary)
i.e. a 100-channel conv (23 GFLOP) followed by a 1x1 512x100 mix (5 GFLOP)
instead of a 512-channel conv (118 GFLOP) -- 4.2x fewer FLOPs.

Precision: single-pass bf16 throughout (1 row/cycle on the PE, same rate as
f32r but half the SBUF/DMA bytes).  Accumulation is fp32 in PSUM.  Measured
end-to-end rel err vs the fp32 reference is ~3.6e-3, comfortably inside the
2e-2 gate; the output is stored bf16 and widened to fp32 on the host.

Schedule: software-pipelined conv(i) / mix(i-1) on the PE.  First and last
tiles are 2 rows tall so the DMA-latency chains (launch+transfer+sem, about
2.2us each way) off the critical path shrink.  Input chunk DMAs and the
batched per-tile output DMAs ride the SP HWDGE queue; weights ride ACT.

Sharding: data-parallel over batch N=16 -> 2 images per core on 8 cores.
The dictionary tap matrices and M^T are small and replicated.
"""

import numpy as np

N_CORES = 8
IMGS_PER_CORE = 2
CIN = 256
COUT = 512
NDICT = 100
H = W = 56
HP = WP = 58  # padded
FREE = 8 * W  # 448, psum tile free size (max tile)
S = 3  # lookup sparsity

# (h0, rows) per image, in global processing order.  img0 runs top-to-bottom
# with full 8-row tiles (a big first tile keeps the tap-consumption cadence
# slower than the weight DMAs land) but its last 8 rows are deferred to the
# very end as two small tiles; img1 does its bottom tiles early.  The final
# four tiles are then all small, so every large output DMA fires while the
# PE is still computing and the program drain is one short chain.
TILES0 = [(8 * t, 8) for t in range(6)]
TILES1 = ([(48, 6), (54, 2)]
          + [(0, 8), (8, 8), (16, 8), (24, 8), (32, 8)]
          + [(40, 6), (46, 2)])
TILES_END0 = [(48, 6), (54, 2)]  # img0's deferred bottom rows
# input row chunks (r0, nr) per image
CHUNKS0 = [(10, 12), (22, 12), (34, 12), (46, 12)]
CHUNKS1 = [(46, 12), (0, 16), (16, 16), (32, 14)]

TRACE = False  # set by test.py to get a profile
_LAST_RESULTS = {}  # test.py reads exec_time_ns from here


def _build_program():
    import concourse.bacc as bacc
    import concourse.mybir as mybir
    import concourse.tile as tile

    f32 = mybir.dt.float32
    bf16 = mybir.dt.bfloat16

    nc = bacc.Bacc("TRN2", target_bir_lowering=False, debug=False)

    # host pre-arranges x into the SBUF layout [cin_in_block, img, cblk, h, w]
    x_d = nc.dram_tensor("x", (128, IMGS_PER_CORE, 2, HP, WP), bf16,
                         kind="ExternalInput")
    w_d = nc.dram_tensor("w", (128, 2 * 9 * NDICT), bf16, kind="ExternalInput")
    m_d = nc.dram_tensor("m", (NDICT, COUT), bf16, kind="ExternalInput")
    # [o_in_block, img, ob, h*w]; host transposes to (i, 512, 56, 56)
    out_d = nc.dram_tensor("out", (128, IMGS_PER_CORE, 4, H * W), bf16,
                           kind="ExternalOutput")

    with tile.TileContext(nc) as tc:
        with (
            tc.tile_pool(name="consts", bufs=1) as consts,
            tc.tile_pool(name="xpool", bufs=1) as xpool,
            tc.tile_pool(name="ypool", bufs=6) as ypool,
            tc.tile_pool(name="opool", bufs=6) as opool,
            tc.tile_pool(name="psum_y", bufs=2, space="PSUM") as psum_y_pool,
            tc.tile_pool(name="psum_o", bufs=6, space="PSUM") as psum_o_pool,
        ):
            # startup-critical loads take the fastest paths: the first three
            # taps ride SWDGE (no HWDGE queueing), the first cb0 x rows are
            # first in the SP HWDGE queue, the rest of the weights ride ACT
            w_sb = consts.tile([128, 2 * 9 * NDICT], bf16)
            x_sb = xpool.tile([128, IMGS_PER_CORE, 2, HP, WP], bf16,
                              tag="x_sb")
            # startup: the first conv runs full-width taps (187ns cadence) so
            # the weight stream keeps ahead: taps 0-8 are the first ACT-queue
            # transfer, taps 9-17 the second, interleaved with the SP queue's
            # first x rows (cb0 needed at tap 0, cb1 at tap 9).
            nc.gpsimd.dma_start(w_sb[:, :6 * NDICT], w_d[:, :6 * NDICT])
            nc.sync.dma_start(x_sb[:, 0, 0, 0:10, :], x_d[:, 0, 0, 0:10, :])
            nc.scalar.dma_start(w_sb[:, 6 * NDICT:], w_d[:, 6 * NDICT:])
            nc.sync.dma_start(x_sb[:, 0, 1, 0:10, :], x_d[:, 0, 1, 0:10, :])
            m_sb = consts.tile([NDICT, COUT], bf16)
            nc.scalar.dma_start(m_sb[:], m_d[:])

            # PE p-state warmup: the Tensor engine clocks 1.2GHz until it has
            # been busy ~3us.  Burn the startup DMA window with tiny matmuls
            # on a zeroed scratch tile so real work starts at full clock.
            scratch = consts.tile([128, 64], bf16)
            nc.vector.memset(scratch[:], 0.0)
            wpsum = psum_y_pool.tile([NDICT, FREE], f32, tag="py")
            for _ in range(46):
                nc.tensor.matmul(wpsum[:64, :64], scratch[:], scratch[:],
                                 start=True, stop=True)

            for img, chunks in ((0, CHUNKS0), (1, CHUNKS1)):
                for r0, nr in chunks:
                    nc.sync.dma_start(
                        x_sb[:, img, :, r0:r0 + nr, :],
                        x_d[:, img, :, r0:r0 + nr, :])

            def emit_conv(img, h0, rows, split=True):
                """One 18-tap accumulation group per tile; the psum->sbuf y
                copy is split column-wise across DVE+ACT so the mix sees half
                the copy latency after the conv's last tap (a stall on the PE
                would also reset its p-state, costing ~2us)."""
                ft = rows * W
                hf = ft // 2
                py = psum_y_pool.tile([NDICT, FREE], f32, tag="py")
                y_sb = ypool.tile([NDICT, FREE], bf16, tag="y")
                for k in range(18):
                    cb, t9 = divmod(k, 9)
                    ti, tj = divmod(t9, 3)
                    tap = slice(k * NDICT, (k + 1) * NDICT)
                    rh = (slice(None), img, cb,
                          slice(h0 + ti, h0 + ti + rows),
                          slice(tj, tj + W))
                    nc.tensor.matmul(
                        py[:, :ft], w_sb[:, tap], x_sb[rh],
                        start=(k == 0), stop=(k == 17))
                if hf:
                    nc.vector.tensor_copy(y_sb[:, :hf], py[:, :hf])
                    nc.scalar.copy(y_sb[:, hf:ft], py[:, hf:ft])
                else:
                    nc.vector.tensor_copy(y_sb[:, :ft], py[:, :ft])
                return py, y_sb

            def emit_mix(py, y_sb, img, h0, rows, dma_eng=None, tail=False):
                ft = rows * W
                o_sb = opool.tile([128, 4, FREE], bf16, tag="o")
                for ob in range(4):
                    obs = slice(ob * 128, (ob + 1) * 128)
                    po = psum_o_pool.tile([128, FREE], f32, tag="po")
                    nc.tensor.matmul(po[:, :ft], m_sb[:, obs], y_sb[:, :ft],
                                     start=True, stop=True)
                    # only DVE and ACT can read PSUM (GPSIMD cannot)
                    if ob % 2 == 0:
                        nc.vector.tensor_copy(o_sb[:, ob, :ft], po[:, :ft])
                    else:
                        nc.scalar.copy(o_sb[:, ob, :ft], po[:, :ft])
                # batched output DMA, on the (mostly idle) SP queue except
                # where the caller spreads the tail across queues
                (dma_eng or nc.sync).dma_start(
                    out_d[:, img, :, h0 * W:h0 * W + ft], o_sb[:, :, :ft])

            # software-pipeline by one tile: PE runs tile i's conv while the
            # copy engines drain tile i-1's psum, so the mix matmuls are
            # ready when PE gets to them
            all_tiles = ([(0,) + t for t in TILES0]
                         + [(1,) + t for t in TILES1]
                         + [(0,) + t for t in TILES_END0])
            n_total = len(all_tiles)
            pending = None
            for i, (img, h0, rows) in enumerate(all_tiles):
                py, y_sb = emit_conv(img, h0, rows, split=(i > 0))
                if pending is not None:
                    # spread the last output DMAs across both HWDGE queues
                    # so the drain chains overlap
                    eng = nc.scalar if i == n_total - 1 else None
                    emit_mix(*pending, dma_eng=eng, tail=(i >= n_total - 2))
                pending = (py, y_sb, img, h0, rows)
            emit_mix(*pending, tail=True)

    nc.compile()
    return nc


_NC_CACHE = None


def kernel(x, dictionary, lookup_indices, lookup_coefficients):
    global _NC_CACHE
    import ml_dtypes
    from concourse import bass_utils

    bf16 = ml_dtypes.bfloat16

    x = np.asarray(x, dtype=np.float32)
    dictionary = np.asarray(dictionary, dtype=np.float32)
    idx = np.asarray(lookup_indices).astype(np.int64)
    coef = np.asarray(lookup_coefficients, dtype=np.float32)

    # M^T[d, o] = sum_s coeff[o, s] * [idx[o, s] == d]
    mt = np.zeros((NDICT, COUT), np.float32)
    np.add.at(mt, (idx.reshape(-1),
                   np.repeat(np.arange(COUT), S)), coef.reshape(-1))

    # wt[c_in_block, (cblk, ti, tj, d)] = dictionary[d, cblk*128+c, ti, tj]
    wt = np.ascontiguousarray(
        dictionary.reshape(NDICT, 2, 128, 3, 3).transpose(2, 1, 3, 4, 0)
    ).reshape(128, 2 * 9 * NDICT)

    # x -> [core, cin_in_block, img, cblk, hp, wp] (the SBUF layout, so the
    # device DMA is a plain contiguous copy)
    xp = np.pad(x, ((0, 0), (0, 0), (1, 1), (1, 1)))
    xp = xp.reshape(N_CORES, IMGS_PER_CORE, 2, 128, HP, WP)
    xp = np.ascontiguousarray(xp.transpose(0, 3, 1, 2, 4, 5))

    if _NC_CACHE is None:
        _NC_CACHE = _build_program()
    nc = _NC_CACHE

    in_maps = [{"x": xp[i].astype(bf16), "w": wt.astype(bf16),
                "m": mt.astype(bf16)} for i in range(N_CORES)]
    try:
        res = bass_utils.run_bass_kernel_spmd(
            nc, in_maps, core_ids=list(range(N_CORES)), trace=TRACE)
    except ModuleNotFoundError:
        # no axon NTFF profile hook in this environment
        res = bass_utils.run_bass_kernel_spmd(
            nc, in_maps, core_ids=list(range(N_CORES)), trace=False)
    _LAST_RESULTS["res"] = res

    # [core, o_in_block, img, ob, hw] -> (16, 512, 56, 56) fp32
    out = np.stack([r["out"] for r in res.results], axis=0)
    out = out.astype(np.float32).transpose(0, 2, 3, 1, 4)
    return np.ascontiguousarray(out).reshape(16, COUT, H, W)


# revision 62
# speedup vs baseline: 2.9519x; 1.0070x over previous
"""LookupConv2d Trainium2 kernel.

Math: out = conv2d(x, W), W[o] = sum_s coeff[o,s] * dictionary[idx[o,s]].
Factorization: W = M @ D where M[o,d] = sum_{s: idx[o,s]=d} coeff[o,s] is a
(512, 100) scatter of the coefficients.  Then
    out = M @ conv2d(x, dictionary)
i.e. a 100-channel conv (23 GFLOP) followed by a 1x1 512x100 mix (5 GFLOP)
instead of a 512-channel conv (118 GFLOP) -- 4.2x fewer FLOPs.

Precision: single-pass bf16 throughout (1 row/cycle on the PE, same rate as
f32r but half the SBUF/DMA bytes).  Accumulation is fp32 in PSUM.  Measured
end-to-end rel err vs the fp32 reference is ~3.6e-3, comfortably inside the
2e-2 gate; the output is stored bf16 and widened to fp32 on the host.

Schedule: software-pipelined conv(i) / mix(i-1) on the PE, with a p-state
warmup so the Tensor engine reaches full clock (2.4GHz) before real work.
Input chunk DMAs and the batched per-tile output DMAs ride the SP HWDGE
queue; weights ride SWDGE+ACT, split so each tap lands before the conv
consumes it.  img0's last rows are processed at the very end as two small
tiles so the final mix/copy/DMA drain chain is short.

Sharding: data-parallel over batch N=16 -> 2 images per core on 8 cores.
The dictionary tap matrices and M^T are small and replicated.
"""

import numpy as np

N_CORES = 8
IMGS_PER_CORE = 2
CIN = 256
COUT = 512
NDICT = 100
H = W = 56
HP = WP = 58  # padded
FREE = 8 * W  # 448, psum tile free size (max tile)
S = 3  # lookup sparsity

# (h0, rows) per image, in global processing order.  img0 runs top-to-bottom
# with full 8-row tiles (a big first tile keeps the tap-consumption cadence
# slower than the weight DMAs land) but its last 8 rows are deferred to the
# very end as two small tiles; img1 does its bottom tiles early.  The final
# four tiles are then all small, so every large output DMA fires while the
# PE is still computing and the program drain is one short chain.
TILES0 = [(8 * t, 8) for t in range(6)]
TILES1 = ([(48, 6), (54, 2)]
          + [(0, 8), (8, 8), (16, 8), (24, 8), (32, 8), (40, 8)])
TILES_END0 = [(48, 6), (54, 2)]  # img0's deferred bottom rows
# input row chunks (r0, nr) per image
CHUNKS0 = [(10, 12), (22, 12), (34, 12), (46, 12)]
CHUNKS1 = [(46, 12), (0, 16), (16, 16), (32, 14)]

TRACE = False  # set by test.py to get a profile
_LAST_RESULTS = {}  # test.py reads exec_time_ns from here


def _build_program():
    import concourse.bacc as bacc
    import concourse.mybir as mybir
    import concourse.tile as tile

    f32 = mybir.dt.float32
    bf16 = mybir.dt.bfloat16

    nc = bacc.Bacc("TRN2", target_bir_lowering=False, debug=False)

    # host pre-arranges x into the SBUF layout [cin_in_block, img, cblk, h, w]
    x_d = nc.dram_tensor("x", (128, IMGS_PER_CORE, 2, HP, WP), bf16,
                         kind="ExternalInput")
    w_d = nc.dram_tensor("w", (128, 2 * 9 * NDICT), bf16, kind="ExternalInput")
    m_d = nc.dram_tensor("m", (NDICT, COUT), bf16, kind="ExternalInput")
    # [o_in_block, img, ob, h*w]; host transposes to (i, 512, 56, 56)
    out_d = nc.dram_tensor("out", (128, IMGS_PER_CORE, 4, H * W), bf16,
                           kind="ExternalOutput")

    with tile.TileContext(nc) as tc:
        with (
            tc.tile_pool(name="consts", bufs=1) as consts,
            tc.tile_pool(name="xpool", bufs=1) as xpool,
            tc.tile_pool(name="ypool", bufs=6) as ypool,
            tc.tile_pool(name="opool", bufs=6) as opool,
            tc.tile_pool(name="psum_y", bufs=2, space="PSUM") as psum_y_pool,
            tc.tile_pool(name="psum_o", bufs=6, space="PSUM") as psum_o_pool,
        ):
            w_sb = consts.tile([128, 2 * 9 * NDICT], bf16)
            x_sb = xpool.tile([128, IMGS_PER_CORE, 2, HP, WP], bf16,
                              tag="x_sb")
            # startup: the first conv consumes tap k at ~T0 + 187*k ns, so
            # the weights stream in pieces that stay ahead of that cadence:
            # taps 0-2 ride SWDGE (third parallel first-slot besides the SP
            # and ACT HWDGE queues), taps 3-8 and 9-17 ride ACT, interleaved
            # with the SP queue's first x rows (cb0 at tap 0, cb1 at tap 9).
            nc.gpsimd.dma_start(w_sb[:, :3 * NDICT], w_d[:, :3 * NDICT])
            nc.sync.dma_start(x_sb[:, 0, 0, 0:10, :], x_d[:, 0, 0, 0:10, :])
            nc.scalar.dma_start(w_sb[:, 3 * NDICT:9 * NDICT],
                                w_d[:, 3 * NDICT:9 * NDICT])
            nc.sync.dma_start(x_sb[:, 0, 1, 0:10, :], x_d[:, 0, 1, 0:10, :])
            nc.scalar.dma_start(w_sb[:, 9 * NDICT:], w_d[:, 9 * NDICT:])
            m_sb = consts.tile([NDICT, COUT], bf16)
            nc.scalar.dma_start(m_sb[:], m_d[:])

            # PE p-state warmup: the Tensor engine clocks 1.2GHz until it has
            # been busy ~3us.  Burn the startup DMA window with tiny matmuls
            # on a zeroed scratch tile so real work starts at full clock.
            scratch = consts.tile([128, 64], bf16)
            nc.vector.memset(scratch[:], 0.0)
            wpsum = psum_y_pool.tile([NDICT, FREE], f32, tag="py")
            for _ in range(46):
                nc.tensor.matmul(wpsum[:64, :64], scratch[:], scratch[:],
                                 start=True, stop=True)

            for img, chunks in ((0, CHUNKS0), (1, CHUNKS1)):
                for r0, nr in chunks:
                    nc.sync.dma_start(
                        x_sb[:, img, :, r0:r0 + nr, :],
                        x_d[:, img, :, r0:r0 + nr, :])

            def emit_conv(img, h0, rows):
                """One 18-tap accumulation group per tile; the psum->sbuf y
                copy is split column-wise across DVE+ACT so the mix sees half
                the copy latency after the conv's last tap (a stall on the PE
                would also reset its p-state, costing ~2us)."""
                ft = rows * W
                hf = ft // 2
                py = psum_y_pool.tile([NDICT, FREE], f32, tag="py")
                y_sb = ypool.tile([NDICT, FREE], bf16, tag="y")
                for k in range(18):
                    cb, t9 = divmod(k, 9)
                    ti, tj = divmod(t9, 3)
                    tap = slice(k * NDICT, (k + 1) * NDICT)
                    rh = (slice(None), img, cb,
                          slice(h0 + ti, h0 + ti + rows),
                          slice(tj, tj + W))
                    nc.tensor.matmul(
                        py[:, :ft], w_sb[:, tap], x_sb[rh],
                        start=(k == 0), stop=(k == 17))
                if hf:
                    nc.vector.tensor_copy(y_sb[:, :hf], py[:, :hf])
                    nc.scalar.copy(y_sb[:, hf:ft], py[:, hf:ft])
                else:
                    nc.vector.tensor_copy(y_sb[:, :ft], py[:, :ft])
                return py, y_sb

            def emit_mix(py, y_sb, img, h0, rows, dma_eng=None):
                ft = rows * W
                o_sb = opool.tile([128, 4, FREE], bf16, tag="o")
                for ob in range(4):
                    obs = slice(ob * 128, (ob + 1) * 128)
                    po = psum_o_pool.tile([128, FREE], f32, tag="po")
                    nc.tensor.matmul(po[:, :ft], m_sb[:, obs], y_sb[:, :ft],
                                     start=True, stop=True)
                    # only DVE and ACT can read PSUM (GPSIMD cannot)
                    if ob % 2 == 0:
                        nc.vector.tensor_copy(o_sb[:, ob, :ft], po[:, :ft])
                    else:
                        nc.scalar.copy(o_sb[:, ob, :ft], po[:, :ft])
                # batched output DMA, on the (mostly idle) SP queue except
                # where the caller spreads the tail across queues
                (dma_eng or nc.sync).dma_start(
                    out_d[:, img, :, h0 * W:h0 * W + ft], o_sb[:, :, :ft])

            # software-pipeline by one tile: PE runs tile i's conv while the
            # copy engines drain tile i-1's psum, so the mix matmuls are
            # ready when PE gets to them
            all_tiles = ([(0,) + t for t in TILES0]
                         + [(1,) + t for t in TILES1]
                         + [(0,) + t for t in TILES_END0])
            n_total = len(all_tiles)
            pending = None
            for i, (img, h0, rows) in enumerate(all_tiles):
                py, y_sb = emit_conv(img, h0, rows)
                if pending is not None:
                    # spread the last output DMAs across both HWDGE queues
                    # so the drain chains overlap
                    eng = nc.scalar if i == n_total - 1 else None
                    emit_mix(*pending, dma_eng=eng)
                pending = (py, y_sb, img, h0, rows)
            emit_mix(*pending)

    nc.compile()
    return nc


_NC_CACHE = None


def kernel(x, dictionary, lookup_indices, lookup_coefficients):
    global _NC_CACHE
    import ml_dtypes
    from concourse import bass_utils

    bf16 = ml_dtypes.bfloat16

    x = np.asarray(x, dtype=np.float32)
    dictionary = np.asarray(dictionary, dtype=np.float32)
    idx = np.asarray(lookup_indices).astype(np.int64)
    coef = np.asarray(lookup_coefficients, dtype=np.float32)

    # M^T[d, o] = sum_s coeff[o, s] * [idx[o, s] == d]
    mt = np.zeros((NDICT, COUT), np.float32)
    np.add.at(mt, (idx.reshape(-1),
                   np.repeat(np.arange(COUT), S)), coef.reshape(-1))

    # wt[c_in_block, (cblk, ti, tj, d)] = dictionary[d, cblk*128+c, ti, tj]
    wt = np.ascontiguousarray(
        dictionary.reshape(NDICT, 2, 128, 3, 3).transpose(2, 1, 3, 4, 0)
    ).reshape(128, 2 * 9 * NDICT)

    # x -> [core, cin_in_block, img, cblk, hp, wp] (the SBUF layout, so the
    # device DMA is a plain contiguous copy)
    xp = np.pad(x, ((0, 0), (0, 0), (1, 1), (1, 1)))
    xp = xp.reshape(N_CORES, IMGS_PER_CORE, 2, 128, HP, WP)
    xp = np.ascontiguousarray(xp.transpose(0, 3, 1, 2, 4, 5))

    if _NC_CACHE is None:
        _NC_CACHE = _build_program()
    nc = _NC_CACHE

    in_maps = [{"x": xp[i].astype(bf16), "w": wt.astype(bf16),
                "m": mt.astype(bf16)} for i in range(N_CORES)]
    try:
        res = bass_utils.run_bass_kernel_spmd(
            nc, in_maps, core_ids=list(range(N_CORES)), trace=TRACE)
    except ModuleNotFoundError:
        # no axon NTFF profile hook in this environment
        res = bass_utils.run_bass_kernel_spmd(
            nc, in_maps, core_ids=list(range(N_CORES)), trace=False)
    _LAST_RESULTS["res"] = res

    # [core, o_in_block, img, ob, hw] -> (16, 512, 56, 56) fp32
    out = np.stack([r["out"] for r in res.results], axis=0)
    out = out.astype(np.float32).transpose(0, 2, 3, 1, 4)
    return np.ascontiguousarray(out).reshape(16, COUT, H, W)
